# revision 13
# baseline (speedup 1.0000x reference)
"""Trainium2 Bass kernel for complex-valued windowed-attention transformer block
(nn_CVSSTL_26027501814276). Data-parallel over batch B=8 across 8 NeuronCores.

Per-core algorithm (batch element of x [4096, 256] complex as r/i planes):
  phase A: LN1 -> PE-transpose to channel-major -> packed qkv matmul producing
    qm=[qr;-qi], qs=[qi;qr], k=[kr;ki] channel-major + v token-major
    -> per-window transposed scores attn^T[k,q] (K stationary on PE)
    -> phase softmax (no max-sub; rowsum via ones-matmul + K=1 broadcast matmul)
    -> block-diagonal U^T over window pairs -> token-major attention output
    -> PE transpose -> proj -> +residual -> x2 (DRAM roundtrip)
  phase B: LN2 -> transpose -> fc1 (ch-major, Lrelu evac) interleaved with fc2
    accumulation (token-major) -> +residual -> output r/i planes.

All complex arithmetic is pre-packed on host into real matmuls over stacked
[re; im] contractions; LN affine, q-scale and rpb gather are folded on host.
"""
import sys

import numpy as np

if '/opt/trn_rl_repo' not in sys.path:
    sys.path.insert(0, '/opt/trn_rl_repo')

B, N, C, H, WS = 8, 4096, 256, 8, 64
HD = C // H
NW = N // WS
SCALE = HD ** -0.5
MLP_HID = 4 * C

NBLK = 8          # token blocks per core
PHASE_A = True    # debug switches
PHASE_B = True
A_LEVEL = 3       # 1=ln+qkv+v, 2=+scores/softmax, 3=full
BLK = 512         # tokens per block
NT = 4            # 128-token tiles per block
WPB = 8           # windows per block
PPB = 4           # window pairs per block


# ---------------------------------------------------------------------------
# host-side weight packing
# ---------------------------------------------------------------------------

def _real_combo(w):
    return np.concatenate([w.real, -w.imag])


def _imag_combo(w):
    return np.concatenate([w.imag, w.real])


def _sbufify(a, p=128):
    K, O = a.shape
    ks = K // p
    return np.ascontiguousarray(
        a.reshape(ks, p, O).transpose(1, 0, 2).reshape(p, ks * O)
    ).astype(np.float32)


def build_packs(inputs):
    norm1_g = np.asarray(inputs['norm1_g'])
    norm1_b = np.asarray(inputs['norm1_b'])
    qkv_w = np.asarray(inputs['qkv_w'])
    qkv_b = np.asarray(inputs['qkv_b'])
    rpb = np.asarray(inputs['rpb_table'])
    proj_w = np.asarray(inputs['proj_w'])
    proj_b = np.asarray(inputs['proj_b'])
    norm2_g = np.asarray(inputs['norm2_g'])
    norm2_b = np.asarray(inputs['norm2_b'])
    fc1_w = np.asarray(inputs['fc1_w'])
    fc1_b = np.asarray(inputs['fc1_b'])
    fc2_w = np.asarray(inputs['fc2_w'])
    fc2_b = np.asarray(inputs['fc2_b'])
    slopes = np.asarray(inputs['prelu_slopes'])

    qkv_w_f = qkv_w * norm1_g[None, :]
    qkv_b_f = qkv_b + qkv_w @ norm1_b
    Wq = qkv_w_f[0:C] * SCALE
    bq = qkv_b_f[0:C] * SCALE
    Wk = qkv_w_f[C:2 * C]
    bk = qkv_b_f[C:2 * C]
    Wv = qkv_w_f[2 * C:3 * C]
    bv = qkv_b_f[2 * C:3 * C]

    cols, bcols = [], []
    for h in range(H):
        rows = slice(h * HD, (h + 1) * HD)
        cols.append(_real_combo(Wq[rows].T))
        bcols.append(bq[rows].real)
        cols.append(-_imag_combo(Wq[rows].T))
        bcols.append(-bq[rows].imag)
    for h in range(H):
        rows = slice(h * HD, (h + 1) * HD)
        cols.append(_imag_combo(Wq[rows].T))
        bcols.append(bq[rows].imag)
        cols.append(_real_combo(Wq[rows].T))
        bcols.append(bq[rows].real)
    for h in range(H):
        rows = slice(h * HD, (h + 1) * HD)
        cols.append(_real_combo(Wk[rows].T))
        bcols.append(bk[rows].real)
        cols.append(_imag_combo(Wk[rows].T))
        bcols.append(bk[rows].imag)
    wbig = np.concatenate(cols, axis=1).real.astype(np.float32)
    bbig = np.concatenate(bcols).astype(np.float32).reshape(12, 128).T.copy()

    vcols, vb = [], []
    for h in range(H):
        rows = slice(h * HD, (h + 1) * HD)
        vcols.append(-_imag_combo(Wv[rows].T))
        vb.append(-bv[rows].imag)
        vcols.append(_real_combo(Wv[rows].T))
        vb.append(bv[rows].real)
        vcols.append(_imag_combo(Wv[rows].T))
        vb.append(bv[rows].imag)
    wv = np.concatenate(vcols, axis=1).real.astype(np.float32)
    bv_bc = np.broadcast_to(
        np.concatenate(vb).astype(np.float32), (128, 768)).copy()

    qi_, ki_ = np.meshgrid(np.arange(WS), np.arange(WS), indexing='ij')
    idx = qi_ - ki_ + WS - 1
    bias_qk = rpb[idx]
    btr = np.zeros((WS, H * WS), np.float32)
    bti = np.zeros((WS, H * WS), np.float32)
    for h in range(H):
        btr[:, h * WS:(h + 1) * WS] = bias_qk[:, :, h].real.T
        bti[:, h * WS:(h + 1) * WS] = bias_qk[:, :, h].imag.T

    pr = np.zeros((512, 512), np.float32)
    for h in range(H):
        cslice = slice(h * HD, (h + 1) * HD)
        er = slice(h * 64, h * 64 + 32)
        ei = slice(h * 64 + 32, h * 64 + 64)
        pr[er, 0:256] = proj_w.real[:, cslice].T
        pr[er, 256:512] = proj_w.imag[:, cslice].T
        pr[ei, 0:256] = -proj_w.imag[:, cslice].T
        pr[ei, 256:512] = proj_w.real[:, cslice].T
    pb_bc = np.broadcast_to(
        np.concatenate([proj_b.real, proj_b.imag]).astype(np.float32),
        (128, 512)).copy()

    fc1_w_f = fc1_w * norm2_g[None, :]
    fc1_b_f = fc1_b + fc1_w @ norm2_b
    wfc1 = np.concatenate(
        [_real_combo(fc1_w_f.T), _imag_combo(fc1_w_f.T)], axis=1
    ).real.astype(np.float32)
    b1 = np.concatenate(
        [fc1_b_f.real, fc1_b_f.imag]).astype(np.float32).reshape(16, 128).T.copy()
    slope_r, slope_i = float(slopes[0]), float(slopes[1])
    oslope = np.array([slope_r] * 8 + [slope_i] * 8, np.float32)
    b1s = b1 * (1.0 - oslope)[None, :]

    w2 = np.zeros((2048, 512), np.float32)
    w2[0:1024, 0:256] = fc2_w.real.T
    w2[0:1024, 256:512] = fc2_w.imag.T
    w2[1024:2048, 0:256] = -fc2_w.imag.T
    w2[1024:2048, 256:512] = fc2_w.real.T
    b2_bc = np.broadcast_to(
        np.concatenate([fc2_b.real, fc2_b.imag]).astype(np.float32),
        (128, 512)).copy()

    return {
        'wbig': _sbufify(wbig), 'bbig': bbig,
        'wv': _sbufify(wv), 'bvbc': bv_bc,
        'btr': btr, 'bti': bti,
        'wproj': _sbufify(pr), 'pbbc': pb_bc,
        'wfc1': _sbufify(wfc1), 'b1': b1, 'b1s': b1s,
        'wfc2': _sbufify(w2), 'b2bc': b2_bc,
        'slope_r': slope_r, 'slope_i': slope_i,
    }


# ---------------------------------------------------------------------------
# device program
# ---------------------------------------------------------------------------

def build_program(slope_r, slope_i):
    import concourse.bacc as bacc
    import concourse.mybir as mybir
    import concourse.tile as tile
    from concourse.masks import make_identity

    f32 = mybir.dt.float32
    AF = mybir.ActivationFunctionType
    ALU = mybir.AluOpType
    AX = mybir.AxisListType

    nc = bacc.Bacc('TRN2', target_bir_lowering=False, debug=False)

    # extra activation-bias constants (same pattern as Bass.__init__ builtins)
    for cval in (1e-5, 1e-18):
        ctns = nc.alloc_sbuf_tensor(f'const-float32-{cval}', [128, 1], f32)
        nc.gpsimd.memset(ctns.ap(), cval)
        nc.const_aps.aps[(f32, cval)] = ctns.ap()
    nc.all_engine_barrier()

    def din(name, shape):
        return nc.dram_tensor(name, shape, f32, kind='ExternalInput')

    xr_d = din('xr', [N, C])
    xi_d = din('xi', [N, C])
    wbig_d = din('wbig', [128, 4 * 1536])
    bbig_d = din('bbig', [128, 12])
    wv_d = din('wv', [128, 4 * 768])
    bvbc_d = din('bvbc', [128, 768])
    btr_d = din('btr', [64, 512])
    bti_d = din('bti', [64, 512])
    wproj_d = din('wproj', [128, 4 * 512])
    pbbc_d = din('pbbc', [128, 512])
    wfc1_d = din('wfc1', [128, 4 * 2048])
    b1_d = din('b1', [128, 16])
    b1s_d = din('b1s', [128, 16])
    wfc2_d = din('wfc2', [128, 16 * 512])
    b2bc_d = din('b2bc', [128, 512])
    outr_d = nc.dram_tensor('outr', [N, C], f32, kind='ExternalOutput')
    outi_d = nc.dram_tensor('outi', [N, C], f32, kind='ExternalOutput')

    with tile.TileContext(nc) as tc:
        with (
            tc.tile_pool(name='static', bufs=1) as stat,
            tc.tile_pool(name='dram', bufs=1, space='DRAM') as dpool,
        ):
            # persistent scratch in DRAM for the phase A -> B handoff
            x2r_d = dpool.tile([N, C], f32)
            x2i_d = dpool.tile([N, C], f32)

            ident = stat.tile([128, 128], f32, tag='ident')
            make_identity(nc, ident)
            ones64 = stat.tile([64, 1], f32, tag='ones64')
            nc.vector.memset(ones64, 1.0)
            ones1 = stat.tile([1, 64], f32, tag='ones1')
            nc.vector.memset(ones1, 1.0)

            btr_s = stat.tile([64, 512], f32, tag='btr')
            nc.sync.dma_start(btr_s[:], btr_d[:])
            bti_s = stat.tile([64, 512], f32, tag='bti')
            nc.sync.dma_start(bti_s[:], bti_d[:])

            # U^T tiles: [128 k(=pair, stacked w0/w1), 8 heads, 128 q(w0|w1)]
            # block-diagonal per head; off-diagonal zeros persist across pairs.
            u2 = {}
            for parity in range(2):
                for comp in ('r', 'i'):
                    t = stat.tile([128, 8, 128], f32, tag=f'u2{comp}{parity}')
                    nc.vector.memset(t, 0.0)
                    u2[(comp, parity)] = t

            # ---------------- phase A ----------------
            if PHASE_A:
              with (
                tc.tile_pool(name='wA', bufs=1) as wA,
                tc.tile_pool(name='xp', bufs=2) as xp,
                tc.tile_pool(name='lnp', bufs=3) as lnp,
                tc.tile_pool(name='zc', bufs=1) as zcp,
                tc.tile_pool(name='qkv', bufs=1) as qkvp,
                tc.tile_pool(name='sm', bufs=2) as smp,
                tc.tile_pool(name='sm1', bufs=1) as smp1,
                tc.tile_pool(name='vp', bufs=5) as vp,
                tc.tile_pool(name='att', bufs=2) as attp,
                tc.tile_pool(name='aotp', bufs=1) as aotp,
                tc.tile_pool(name='psMM', bufs=3, space='PSUM') as psMM,
                tc.tile_pool(name='psSC', bufs=4, space='PSUM') as psSC,
                tc.tile_pool(name='psSM', bufs=1, space='PSUM') as psSM,
            ):
                wbig_s = wA.tile([128, 4, 1536], f32, tag='wbig')
                nc.sync.dma_start(wbig_s[:], wbig_d[:].rearrange('p (k o) -> p k o', k=4))
                bbig_s = wA.tile([128, 12], f32, tag='bbig')
                nc.sync.dma_start(bbig_s[:], bbig_d[:])
                wv_s = wA.tile([128, 4, 768], f32, tag='wv')
                nc.sync.dma_start(wv_s[:], wv_d[:].rearrange('p (k o) -> p k o', k=4))
                bvbc_s = wA.tile([128, 768], f32, tag='bvbc')
                nc.sync.dma_start(bvbc_s[:], bvbc_d[:])
                wproj_s = wA.tile([128, 4, 512], f32, tag='wproj')
                nc.sync.dma_start(wproj_s[:], wproj_d[:].rearrange('p (k o) -> p k o', k=4))
                pbbc_s = wA.tile([128, 512], f32, tag='pbbc')
                nc.sync.dma_start(pbbc_s[:], pbbc_d[:])

                for blk in range(NBLK):
                    t0 = blk * BLK
                    # ---- load x block + LN1 + transpose to channel-major
                    x_t = {}
                    for comp, xd in (('r', xr_d), ('i', xi_d)):
                        xt = xp.tile([128, NT, C], f32, tag=f'x{comp}')
                        nc.sync.dma_start(
                            xt[:], xd[t0:t0 + BLK, :].rearrange(
                                '(t p) c -> p t c', p=128))
                        x_t[comp] = xt
                    zcat = zcp.tile([128, 4, BLK], f32, tag='zcat')
                    for t in range(NT):
                        tb = psMM.tile([128, 512], f32, tag='mm')
                        for ci, comp in enumerate(('r', 'i')):
                            x_ap = x_t[comp][:, t, :]
                            ssum = lnp.tile([128, 1], f32, tag='ln_s')
                            nc.vector.reduce_sum(out=ssum, in_=x_ap, axis=AX.X)
                            nm = lnp.tile([128, 1], f32, tag='ln_nm')
                            nc.vector.tensor_scalar_mul(nm, ssum, -1.0 / C)
                            sq = lnp.tile([128, C], f32, tag='ln_sq')
                            ssq = lnp.tile([128, 1], f32, tag='ln_ssq')
                            nc.scalar.activation(sq, x_ap, AF.Square,
                                                 bias=nm, accum_out=ssq)
                            sd = lnp.tile([128, 1], f32, tag='ln_sd')
                            nc.scalar.activation(sd, ssq, AF.Sqrt,
                                                 bias=1e-5, scale=1.0 / C)
                            rstd = lnp.tile([128, 1], f32, tag='ln_rstd')
                            nc.vector.reciprocal(rstd, sd)
                            z = lnp.tile([128, C], f32, tag=f'ln_z{comp}')
                            nc.vector.tensor_scalar(
                                z, x_ap, nm, rstd, op0=ALU.add, op1=ALU.mult)
                            for j in range(2):
                                nc.tensor.transpose(
                                    tb[:, (ci * 2 + j) * 128:(ci * 2 + j + 1) * 128],
                                    z[:, j * 128:(j + 1) * 128], ident)
                        nc.any.tensor_copy(
                            zcat[:, :, t * 128:(t + 1) * 128],
                            tb[:].rearrange('p (s q) -> p s q', s=4))

                    # ---- qkv channel-major sections (qm, qs, k)
                    sec = {}
                    for si, name in enumerate(('qm', 'qs', 'k')):
                        st = qkvp.tile([128, 4, BLK], f32, tag=f'sec_{name}')
                        sec[name] = st
                        for hp in range(4):
                            osl = si * 4 + hp
                            ps = psMM.tile([128, 512], f32, tag='mm')
                            for ks in range(4):
                                nc.tensor.matmul(
                                    ps[:],
                                    wbig_s[:, ks, osl * 128:(osl + 1) * 128],
                                    zcat[:, ks, :],
                                    start=(ks == 0), stop=(ks == 3))
                            nc.scalar.activation(
                                st[:, hp, :], ps[:], AF.Identity,
                                bias=bbig_s[:, osl:osl + 1])

                    # ---- v token-major [128 tok, 768 (h:(eineg,er,ei))]
                    vtok = []
                    for t in range(NT):
                        vt = vp.tile([128, 768], f32, tag='vtok')
                        vtok.append(vt)
                        psa = psMM.tile([128, 512], f32, tag='mm')
                        psb = psMM.tile([128, 512], f32, tag='mm')
                        for ks in range(4):
                            lhs = zcat[:, ks, t * 128:(t + 1) * 128]
                            nc.tensor.matmul(psa[:], lhs, wv_s[:, ks, 0:512],
                                             start=(ks == 0), stop=(ks == 3))
                            nc.tensor.matmul(psb[:, 0:256], lhs,
                                             wv_s[:, ks, 512:768],
                                             start=(ks == 0), stop=(ks == 3))
                        nc.vector.tensor_add(vt[:, 0:512], psa[:],
                                             bvbc_s[:, 0:512])
                        nc.vector.tensor_add(vt[:, 512:768], psb[:, 0:256],
                                             bvbc_s[:, 512:768])

                    # ---- attention per window
                    for p in range(PPB if A_LEVEL >= 2 else 0):
                        parity = p % 2
                        u2r = u2[('r', parity)]
                        u2i = u2[('i', parity)]
                        for half in range(2):
                            w = p * 2 + half
                            c0 = w * 64  # block-local token offset of window
                            # per-head full-bank psum tiles (matmul psum dst
                            # must sit at a bank start on HW); gather+bias
                            # into SBUF [64, (h q)] immediately.
                            arp = smp.tile([64, 512], f32, tag='arp')
                            aip = smp.tile([64, 512], f32, tag='aip')
                            for h in range(H):
                                hp, off = h // 2, (h % 2) * 64
                                ksl = sec['k'][off:off + 64, hp, c0:c0 + 64]
                                hs = slice(h * 64, (h + 1) * 64)
                                bank_r = psSC.tile([64, 512], f32, tag='sc',
                                                   name=f'scr{blk}_{w}_{h}')
                                nc.tensor.matmul(
                                    bank_r[:, 0:64], ksl,
                                    sec['qm'][off:off + 64, hp, c0:c0 + 64],
                                    start=True, stop=True)
                                bank_i = psSC.tile([64, 512], f32, tag='sc',
                                                   name=f'sci{blk}_{w}_{h}')
                                nc.tensor.matmul(
                                    bank_i[:, 0:64], ksl,
                                    sec['qs'][off:off + 64, hp, c0:c0 + 64],
                                    start=True, stop=True)
                                nc.vector.tensor_add(
                                    arp[:, hs], bank_r[:, 0:64], btr_s[:, hs])
                                nc.vector.tensor_add(
                                    aip[:, hs], bank_i[:, 0:64], bti_s[:, hs])
                            # phase softmax (transposed layout [k, (h q)])
                            sqr = smp1.tile([64, 512], f32, tag='sqr')
                            nc.scalar.activation(sqr, arp, AF.Square)
                            sqi = smp1.tile([64, 512], f32, tag='sqi')
                            nc.scalar.activation(sqi, aip, AF.Square)
                            m2 = smp1.tile([64, 512], f32, tag='m2')
                            nc.vector.tensor_add(m2, sqr, sqi)
                            mag = smp1.tile([64, 512], f32, tag='mag')
                            nc.scalar.activation(mag, m2, AF.Sqrt, bias=1e-18)
                            e = smp.tile([64, 512], f32, tag='e')
                            nc.scalar.activation(e, mag, AF.Exp)
                            s1 = psSM.tile([1, 512], f32, tag='sm')
                            nc.tensor.matmul(s1[:], ones64[:], e[:],
                                             start=True, stop=True)
                            s1s = smp.tile([1, 512], f32, tag='s1s')
                            nc.scalar.copy(s1s, s1[:])
                            sbc = psSM.tile([64, 512], f32, tag='sm')
                            nc.tensor.matmul(sbc[:], ones1[:], s1s[:],
                                             start=True, stop=True)
                            t2 = smp1.tile([64, 512], f32, tag='t2')
                            nc.vector.tensor_mul(t2, mag, sbc[:])
                            inv2 = smp1.tile([64, 512], f32, tag='inv2')
                            nc.vector.reciprocal(inv2, t2)
                            g2 = smp1.tile([64, 512], f32, tag='g2')
                            nc.vector.tensor_mul(g2, e, inv2)
                            # U^T writes into block-diagonal slices
                            po = half * 64
                            arp3 = arp[:].rearrange('p (h q) -> p h q', h=8)
                            aip3 = aip[:].rearrange('p (h q) -> p h q', h=8)
                            g23 = g2[:].rearrange('p (h q) -> p h q', h=8)
                            nc.vector.tensor_mul(
                                u2r[po:po + 64, :, po:po + 64], arp3, g23)
                            nc.vector.tensor_mul(
                                u2i[po:po + 64, :, po:po + 64], aip3, g23)

                        # ---- attention output, token-major [128 q, 512]
                        if A_LEVEL < 3:
                            continue
                        aos = attp.tile([128, 512], f32, tag='aos')
                        for h in range(H):
                            vs = vtok[p]
                            aoh = psSC.tile([128, 64], f32, tag='sc',
                                            name=f'ao{blk}_{p}_{h}')
                            nc.tensor.matmul(
                                aoh[:],
                                u2r[:, h, :],
                                vs[:, h * 96 + 32:h * 96 + 96],
                                start=True, stop=False)
                            nc.tensor.matmul(
                                aoh[:],
                                u2i[:, h, :],
                                vs[:, h * 96:h * 96 + 64],
                                start=False, stop=True)
                            nc.any.tensor_copy(
                                aos[:, h * 64:(h + 1) * 64], aoh[:])
                        # transpose to channel-major
                        tb2 = psMM.tile([128, 512], f32, tag='mm')
                        for j in range(4):
                            nc.tensor.transpose(
                                tb2[:, j * 128:(j + 1) * 128],
                                aos[:, j * 128:(j + 1) * 128], ident)
                        if p == 0:
                            aot = aotp.tile([128, 4, BLK], f32, tag='aot')
                        nc.any.tensor_copy(
                            aot[:, :, p * 128:(p + 1) * 128],
                            tb2[:].rearrange('p (s q) -> p s q', s=4))

                    # ---- proj (token-major out) + bias + residual -> x2
                    for t in range(NT if A_LEVEL >= 3 else 0):
                        yps = psMM.tile([128, 512], f32, tag='mm')
                        for ks in range(4):
                            nc.tensor.matmul(
                                yps[:],
                                aot[:, ks, t * 128:(t + 1) * 128],
                                wproj_s[:, ks, :],
                                start=(ks == 0), stop=(ks == 3))
                        for ci, (comp, x2d) in enumerate(
                                (('r', x2r_d), ('i', x2i_d))):
                            tmp = attp.tile([128, C], f32, tag='x2tmp')
                            nc.vector.tensor_add(
                                tmp, yps[:, ci * 256:(ci + 1) * 256],
                                pbbc_s[:, ci * 256:(ci + 1) * 256])
                            x2sb = attp.tile([128, C], f32, tag='x2sb')
                            nc.vector.tensor_add(x2sb, tmp,
                                                 x_t[comp][:, t, :])
                            nc.sync.dma_start(
                                x2d[t0 + t * 128:t0 + (t + 1) * 128, :],
                                x2sb[:])

            # ---------------- phase B (MLP) ----------------
            if PHASE_B:
              with (
                tc.tile_pool(name='wB', bufs=1) as wB,
                tc.tile_pool(name='xp2', bufs=2) as xp2,
                tc.tile_pool(name='lnp2', bufs=3) as lnp2,
                tc.tile_pool(name='zc2', bufs=2) as zcp2,
                tc.tile_pool(name='hp', bufs=3) as hp_,
                tc.tile_pool(name='ob', bufs=4) as obp,
                tc.tile_pool(name='psF', bufs=2, space='PSUM') as psF,
                tc.tile_pool(name='psY2', bufs=4, space='PSUM') as psY2,
            ):
                wfc1_s = wB.tile([128, 4, 2048], f32, tag='wfc1')
                nc.sync.dma_start(wfc1_s[:], wfc1_d[:].rearrange('p (k o) -> p k o', k=4))
                b1_s = wB.tile([128, 16], f32, tag='b1')
                nc.sync.dma_start(b1_s[:], b1_d[:])
                b1s_s = wB.tile([128, 16], f32, tag='b1s')
                nc.sync.dma_start(b1s_s[:], b1s_d[:])
                wfc2_s = wB.tile([128, 16, 512], f32, tag='wfc2')
                nc.sync.dma_start(wfc2_s[:], wfc2_d[:].rearrange('p (k o) -> p k o', k=16))
                b2bc_s = wB.tile([128, 512], f32, tag='b2bc')
                nc.sync.dma_start(b2bc_s[:], b2bc_d[:])

                for blk in range(NBLK):
                    t0 = blk * BLK
                    x2_t = {}
                    for comp, x2d in (('r', x2r_d), ('i', x2i_d)):
                        xt = xp2.tile([128, NT, C], f32, tag=f'x2{comp}')
                        nc.sync.dma_start(
                            xt[:], x2d[t0:t0 + BLK, :].rearrange(
                                '(t p) c -> p t c', p=128))
                        x2_t[comp] = xt
                    z2cat = zcp2.tile([128, 4, BLK], f32, tag='z2cat')
                    for t in range(NT):
                        tb = psF.tile([128, 512], f32, tag='f')
                        for ci, comp in enumerate(('r', 'i')):
                            x_ap = x2_t[comp][:, t, :]
                            ssum = lnp2.tile([128, 1], f32, tag='ln_s')
                            nc.vector.reduce_sum(out=ssum, in_=x_ap, axis=AX.X)
                            nm = lnp2.tile([128, 1], f32, tag='ln_nm')
                            nc.vector.tensor_scalar_mul(nm, ssum, -1.0 / C)
                            sq = lnp2.tile([128, C], f32, tag='ln_sq')
                            ssq = lnp2.tile([128, 1], f32, tag='ln_ssq')
                            nc.scalar.activation(sq, x_ap, AF.Square,
                                                 bias=nm, accum_out=ssq)
                            sd = lnp2.tile([128, 1], f32, tag='ln_sd')
                            nc.scalar.activation(sd, ssq, AF.Sqrt,
                                                 bias=1e-5, scale=1.0 / C)
                            rstd = lnp2.tile([128, 1], f32, tag='ln_rstd')
                            nc.vector.reciprocal(rstd, sd)
                            z = lnp2.tile([128, C], f32, tag=f'ln_z{comp}')
                            nc.vector.tensor_scalar(
                                z, x_ap, nm, rstd, op0=ALU.add, op1=ALU.mult)
                            for j in range(2):
                                nc.tensor.transpose(
                                    tb[:, (ci * 2 + j) * 128:(ci * 2 + j + 1) * 128],
                                    z[:, j * 128:(j + 1) * 128], ident)
                        nc.any.tensor_copy(
                            z2cat[:, :, t * 128:(t + 1) * 128],
                            tb[:].rearrange('p (s q) -> p s q', s=4))

                    # fc1 (ch-major, Lrelu evac) interleaved with fc2 accum
                    y2ps = [psY2.tile([128, 512], f32, tag='y2',
                                      name=f'y2ps_{blk}_{t}')
                            for t in range(NT)]
                    for j in range(16):
                        fps = psF.tile([128, 512], f32, tag='f')
                        for ks in range(4):
                            nc.tensor.matmul(
                                fps[:],
                                wfc1_s[:, ks, j * 128:(j + 1) * 128],
                                z2cat[:, ks, :],
                                start=(ks == 0), stop=(ks == 3))
                        slope = slope_r if j < 8 else slope_i
                        a_t = hp_.tile([128, 512], f32, tag='pre')
                        nc.scalar.activation(a_t[:], fps[:], AF.Identity,
                                             bias=b1_s[:, j:j + 1])
                        r_t = hp_.tile([128, 512], f32, tag='rel')
                        nc.scalar.activation(r_t[:], fps[:], AF.Relu,
                                             bias=b1s_s[:, j:j + 1],
                                             scale=1.0 - slope)
                        hj = hp_.tile([128, 512], f32, tag='hj')
                        nc.vector.scalar_tensor_tensor(
                            hj[:], a_t[:], slope, r_t[:],
                            op0=ALU.mult, op1=ALU.add)
                        for t in range(NT):
                            nc.tensor.matmul(
                                y2ps[t][:],
                                hj[:, t * 128:(t + 1) * 128],
                                wfc2_s[:, j, :],
                                start=(j == 0), stop=(j == 15))
                    for t in range(NT):
                        for ci, (comp, od) in enumerate(
                                (('r', outr_d), ('i', outi_d))):
                            tmp = obp.tile([128, C], f32, tag='otmp')
                            nc.vector.tensor_add(
                                tmp, y2ps[t][:, ci * 256:(ci + 1) * 256],
                                b2bc_s[:, ci * 256:(ci + 1) * 256])
                            osb = obp.tile([128, C], f32, tag='osb')
                            nc.vector.tensor_add(osb, tmp, x2_t[comp][:, t, :])
                            nc.sync.dma_start(
                                od[t0 + t * 128:t0 + (t + 1) * 128, :], osb[:])

    nc.compile()
    return nc


_CACHE = {}


def _get_program(slope_r, slope_i):
    key = (slope_r, slope_i)
    if key not in _CACHE:
        _CACHE[key] = build_program(slope_r, slope_i)
    return _CACHE[key]


def _run(inputs, trace=False):
    from concourse.bass_utils import run_bass_kernel_spmd

    pk = build_packs(inputs)
    nc = _get_program(pk['slope_r'], pk['slope_i'])
    xr = np.asarray(inputs['x_real'], np.float32)
    xi = np.asarray(inputs['x_imag'], np.float32)
    shared = {k: pk[k] for k in ('wbig', 'bbig', 'wv', 'bvbc', 'btr', 'bti',
                                 'wproj', 'pbbc', 'wfc1', 'b1', 'b1s',
                                 'wfc2', 'b2bc')}
    in_maps = []
    for b in range(B):
        m = dict(shared)
        m['xr'] = np.ascontiguousarray(xr[b])
        m['xi'] = np.ascontiguousarray(xi[b])
        in_maps.append(m)
    try:
        res = run_bass_kernel_spmd(nc, in_maps, core_ids=list(range(B)),
                                   trace=trace)
    except ModuleNotFoundError:
        # NTFF profiling hook unavailable in this container; run untraced.
        res = run_bass_kernel_spmd(nc, in_maps, core_ids=list(range(B)),
                                   trace=False)
    out = np.empty((2, B, N, C), np.float32)
    for b in range(B):
        out[0, b] = res.results[b]['outr']
        out[1, b] = res.results[b]['outi']
    return out, res.exec_time_ns


def kernel(**inputs):
    out, _ = _run(inputs)
    return out


# revision 15
# speedup vs baseline: 1.8068x; 1.8068x over previous
"""Trainium2 Bass kernel for complex-valued windowed-attention transformer block
(nn_CVSSTL_26027501814276). Data-parallel over batch B=8 across 8 NeuronCores.

Per-core algorithm (batch element of x [4096, 256] complex as r/i planes):
  phase A: LN1 -> PE-transpose to channel-major -> packed qkv matmul producing
    qm=[qr;-qi], qs=[qi;qr], k=[kr;ki] channel-major + v token-major
    -> per-window transposed scores attn^T[k,q] (K stationary on PE)
    -> phase softmax (no max-sub; rowsum via ones-matmul + K=1 broadcast matmul)
    -> block-diagonal U^T over window pairs -> token-major attention output
    -> PE transpose -> proj -> +residual -> x2 (DRAM roundtrip)
  phase B: LN2 -> transpose -> fc1 (ch-major, Lrelu evac) interleaved with fc2
    accumulation (token-major) -> +residual -> output r/i planes.

All complex arithmetic is pre-packed on host into real matmuls over stacked
[re; im] contractions; LN affine, q-scale and rpb gather are folded on host.
"""
import sys

import numpy as np

if '/opt/trn_rl_repo' not in sys.path:
    sys.path.insert(0, '/opt/trn_rl_repo')

B, N, C, H, WS = 8, 4096, 256, 8, 64
HD = C // H
NW = N // WS
SCALE = HD ** -0.5
MLP_HID = 4 * C

NBLK = 8          # token blocks per core
PHASE_A = True    # debug switches
PHASE_B = True
A_LEVEL = 3       # 1=ln+qkv+v, 2=+scores/softmax, 3=full
BLK = 512         # tokens per block
NT = 4            # 128-token tiles per block
WPB = 8           # windows per block
PPB = 4           # window pairs per block


# ---------------------------------------------------------------------------
# host-side weight packing
# ---------------------------------------------------------------------------

def _real_combo(w):
    return np.concatenate([w.real, -w.imag])


def _imag_combo(w):
    return np.concatenate([w.imag, w.real])


def _sbufify(a, p=128):
    K, O = a.shape
    ks = K // p
    return np.ascontiguousarray(
        a.reshape(ks, p, O).transpose(1, 0, 2).reshape(p, ks * O)
    ).astype(np.float32)


def build_packs(inputs):
    norm1_g = np.asarray(inputs['norm1_g'])
    norm1_b = np.asarray(inputs['norm1_b'])
    qkv_w = np.asarray(inputs['qkv_w'])
    qkv_b = np.asarray(inputs['qkv_b'])
    rpb = np.asarray(inputs['rpb_table'])
    proj_w = np.asarray(inputs['proj_w'])
    proj_b = np.asarray(inputs['proj_b'])
    norm2_g = np.asarray(inputs['norm2_g'])
    norm2_b = np.asarray(inputs['norm2_b'])
    fc1_w = np.asarray(inputs['fc1_w'])
    fc1_b = np.asarray(inputs['fc1_b'])
    fc2_w = np.asarray(inputs['fc2_w'])
    fc2_b = np.asarray(inputs['fc2_b'])
    slopes = np.asarray(inputs['prelu_slopes'])

    qkv_w_f = qkv_w * norm1_g[None, :]
    qkv_b_f = qkv_b + qkv_w @ norm1_b
    Wq = qkv_w_f[0:C] * SCALE
    bq = qkv_b_f[0:C] * SCALE
    Wk = qkv_w_f[C:2 * C]
    bk = qkv_b_f[C:2 * C]
    Wv = qkv_w_f[2 * C:3 * C]
    bv = qkv_b_f[2 * C:3 * C]

    cols, bcols = [], []
    for h in range(H):
        rows = slice(h * HD, (h + 1) * HD)
        cols.append(_real_combo(Wq[rows].T))
        bcols.append(bq[rows].real)
        cols.append(-_imag_combo(Wq[rows].T))
        bcols.append(-bq[rows].imag)
    for h in range(H):
        rows = slice(h * HD, (h + 1) * HD)
        cols.append(_imag_combo(Wq[rows].T))
        bcols.append(bq[rows].imag)
        cols.append(_real_combo(Wq[rows].T))
        bcols.append(bq[rows].real)
    for h in range(H):
        rows = slice(h * HD, (h + 1) * HD)
        cols.append(_real_combo(Wk[rows].T))
        bcols.append(bk[rows].real)
        cols.append(_imag_combo(Wk[rows].T))
        bcols.append(bk[rows].imag)
    wbig = np.concatenate(cols, axis=1).real.astype(np.float32)
    bbig = np.concatenate(bcols).astype(np.float32).reshape(12, 128).T.copy()

    vcols, vb = [], []
    for h in range(H):
        rows = slice(h * HD, (h + 1) * HD)
        vcols.append(-_imag_combo(Wv[rows].T))
        vb.append(-bv[rows].imag)
        vcols.append(_real_combo(Wv[rows].T))
        vb.append(bv[rows].real)
        vcols.append(_imag_combo(Wv[rows].T))
        vb.append(bv[rows].imag)
    wv = np.concatenate(vcols, axis=1).real.astype(np.float32)
    bv_bc = np.broadcast_to(
        np.concatenate(vb).astype(np.float32), (128, 768)).copy()

    qi_, ki_ = np.meshgrid(np.arange(WS), np.arange(WS), indexing='ij')
    idx = qi_ - ki_ + WS - 1
    bias_qk = rpb[idx]
    btr = np.zeros((WS, H * WS), np.float32)
    bti = np.zeros((WS, H * WS), np.float32)
    for h in range(H):
        btr[:, h * WS:(h + 1) * WS] = bias_qk[:, :, h].real.T
        bti[:, h * WS:(h + 1) * WS] = bias_qk[:, :, h].imag.T

    pr = np.zeros((512, 512), np.float32)
    for h in range(H):
        cslice = slice(h * HD, (h + 1) * HD)
        er = slice(h * 64, h * 64 + 32)
        ei = slice(h * 64 + 32, h * 64 + 64)
        pr[er, 0:256] = proj_w.real[:, cslice].T
        pr[er, 256:512] = proj_w.imag[:, cslice].T
        pr[ei, 0:256] = -proj_w.imag[:, cslice].T
        pr[ei, 256:512] = proj_w.real[:, cslice].T
    pb_bc = np.broadcast_to(
        np.concatenate([proj_b.real, proj_b.imag]).astype(np.float32),
        (128, 512)).copy()

    fc1_w_f = fc1_w * norm2_g[None, :]
    fc1_b_f = fc1_b + fc1_w @ norm2_b
    wfc1 = np.concatenate(
        [_real_combo(fc1_w_f.T), _imag_combo(fc1_w_f.T)], axis=1
    ).real.astype(np.float32)
    b1 = np.concatenate(
        [fc1_b_f.real, fc1_b_f.imag]).astype(np.float32).reshape(16, 128).T.copy()
    slope_r, slope_i = float(slopes[0]), float(slopes[1])
    oslope = np.array([slope_r] * 8 + [slope_i] * 8, np.float32)
    b1s = b1 * (1.0 - oslope)[None, :]

    w2 = np.zeros((2048, 512), np.float32)
    w2[0:1024, 0:256] = fc2_w.real.T
    w2[0:1024, 256:512] = fc2_w.imag.T
    w2[1024:2048, 0:256] = -fc2_w.imag.T
    w2[1024:2048, 256:512] = fc2_w.real.T
    b2_bc = np.broadcast_to(
        np.concatenate([fc2_b.real, fc2_b.imag]).astype(np.float32),
        (128, 512)).copy()

    return {
        'wbig': _sbufify(wbig), 'bbig': bbig,
        'wv': _sbufify(wv), 'bvbc': bv_bc,
        'btr': btr, 'bti': bti,
        'wproj': _sbufify(pr), 'pbbc': pb_bc,
        'wfc1': _sbufify(wfc1), 'b1': b1, 'b1s': b1s,
        'wfc2': _sbufify(w2), 'b2bc': b2_bc,
        'slope_r': slope_r, 'slope_i': slope_i,
    }


# ---------------------------------------------------------------------------
# device program
# ---------------------------------------------------------------------------

def build_program(slope_r, slope_i):
    import concourse.bacc as bacc
    import concourse.mybir as mybir
    import concourse.tile as tile
    from concourse.masks import make_identity

    f32 = mybir.dt.float32
    f32r = mybir.dt.float32r
    AF = mybir.ActivationFunctionType
    ALU = mybir.AluOpType
    AX = mybir.AxisListType

    nc = bacc.Bacc('TRN2', target_bir_lowering=False, debug=False)

    # extra activation-bias constants (same pattern as Bass.__init__ builtins)
    for cval in (1e-5, 1e-18):
        ctns = nc.alloc_sbuf_tensor(f'const-float32-{cval}', [128, 1], f32)
        nc.gpsimd.memset(ctns.ap(), cval)
        nc.const_aps.aps[(f32, cval)] = ctns.ap()
    nc.all_engine_barrier()

    def din(name, shape):
        return nc.dram_tensor(name, shape, f32, kind='ExternalInput')

    xr_d = din('xr', [N, C])
    xi_d = din('xi', [N, C])
    wbig_d = din('wbig', [128, 4 * 1536])
    bbig_d = din('bbig', [128, 12])
    wv_d = din('wv', [128, 4 * 768])
    bvbc_d = din('bvbc', [128, 768])
    btr_d = din('btr', [64, 512])
    bti_d = din('bti', [64, 512])
    wproj_d = din('wproj', [128, 4 * 512])
    pbbc_d = din('pbbc', [128, 512])
    wfc1_d = din('wfc1', [128, 4 * 2048])
    b1_d = din('b1', [128, 16])
    b1s_d = din('b1s', [128, 16])
    wfc2_d = din('wfc2', [128, 16 * 512])
    b2bc_d = din('b2bc', [128, 512])
    outr_d = nc.dram_tensor('outr', [N, C], f32, kind='ExternalOutput')
    outi_d = nc.dram_tensor('outi', [N, C], f32, kind='ExternalOutput')

    with tile.TileContext(nc) as tc:
        with (
            tc.tile_pool(name='static', bufs=1) as stat,
            tc.tile_pool(name='dram', bufs=1, space='DRAM') as dpool,
        ):
            # persistent scratch in DRAM for the phase A -> B handoff
            x2r_d = dpool.tile([N, C], f32)
            x2i_d = dpool.tile([N, C], f32)

            ident = stat.tile([128, 128], f32, tag='ident')
            make_identity(nc, ident)
            ones64 = stat.tile([64, 1], f32, tag='ones64')
            nc.vector.memset(ones64, 1.0)
            ones1 = stat.tile([1, 64], f32, tag='ones1')
            nc.vector.memset(ones1, 1.0)

            btr_s = stat.tile([64, 512], f32, tag='btr')
            nc.sync.dma_start(btr_s[:], btr_d[:])
            bti_s = stat.tile([64, 512], f32, tag='bti')
            nc.sync.dma_start(bti_s[:], bti_d[:])

            # U^T tiles: [128 k(=pair, stacked w0/w1), 8 heads, 128 q(w0|w1)]
            # block-diagonal per head; off-diagonal zeros persist across pairs.
            u2 = {}
            for parity in range(2):
                for comp in ('r', 'i'):
                    t = stat.tile([128, 8, 128], f32, tag=f'u2{comp}{parity}')
                    nc.vector.memset(t, 0.0)
                    u2[(comp, parity)] = t

            # ---------------- phase A ----------------
            if PHASE_A:
              with (
                tc.tile_pool(name='wA', bufs=1) as wA,
                tc.tile_pool(name='xp', bufs=2) as xp,
                tc.tile_pool(name='lnp', bufs=3) as lnp,
                tc.tile_pool(name='zc', bufs=1) as zcp,
                tc.tile_pool(name='qkv', bufs=1) as qkvp,
                tc.tile_pool(name='sm', bufs=2) as smp,
                tc.tile_pool(name='sm1', bufs=1) as smp1,
                tc.tile_pool(name='vp', bufs=5) as vp,
                tc.tile_pool(name='att', bufs=2) as attp,
                tc.tile_pool(name='aotp', bufs=1) as aotp,
                tc.tile_pool(name='psMM', bufs=3, space='PSUM') as psMM,
                tc.tile_pool(name='psSC', bufs=4, space='PSUM') as psSC,
                tc.tile_pool(name='psSM', bufs=1, space='PSUM') as psSM,
            ):
                wbig_s = wA.tile([128, 4, 1536], f32r, tag='wbig')
                nc.sync.dma_start(wbig_s[:], wbig_d[:].rearrange('p (k o) -> p k o', k=4).bitcast(f32r))
                bbig_s = wA.tile([128, 12], f32, tag='bbig')
                nc.sync.dma_start(bbig_s[:], bbig_d[:])
                wv_s = wA.tile([128, 4, 768], f32r, tag='wv')
                nc.sync.dma_start(wv_s[:], wv_d[:].rearrange('p (k o) -> p k o', k=4).bitcast(f32r))
                bvbc_s = wA.tile([128, 768], f32, tag='bvbc')
                nc.sync.dma_start(bvbc_s[:], bvbc_d[:])
                wproj_s = wA.tile([128, 4, 512], f32r, tag='wproj')
                nc.sync.dma_start(wproj_s[:], wproj_d[:].rearrange('p (k o) -> p k o', k=4).bitcast(f32r))
                pbbc_s = wA.tile([128, 512], f32, tag='pbbc')
                nc.sync.dma_start(pbbc_s[:], pbbc_d[:])

                for blk in range(NBLK):
                    t0 = blk * BLK
                    # ---- load x block + LN1 + transpose to channel-major
                    x_t = {}
                    for comp, xd in (('r', xr_d), ('i', xi_d)):
                        xt = xp.tile([128, NT, C], f32, tag=f'x{comp}')
                        nc.sync.dma_start(
                            xt[:], xd[t0:t0 + BLK, :].rearrange(
                                '(t p) c -> p t c', p=128))
                        x_t[comp] = xt
                    zcat = zcp.tile([128, 4, BLK], f32r, tag='zcat')
                    for t in range(NT):
                        tb = psMM.tile([128, 512], f32, tag='mm')
                        for ci, comp in enumerate(('r', 'i')):
                            x_ap = x_t[comp][:, t, :]
                            ssum = lnp.tile([128, 1], f32, tag='ln_s')
                            nc.vector.reduce_sum(out=ssum, in_=x_ap, axis=AX.X)
                            nm = lnp.tile([128, 1], f32, tag='ln_nm')
                            nc.vector.tensor_scalar_mul(nm, ssum, -1.0 / C)
                            sq = lnp.tile([128, C], f32, tag='ln_sq')
                            ssq = lnp.tile([128, 1], f32, tag='ln_ssq')
                            nc.scalar.activation(sq, x_ap, AF.Square,
                                                 bias=nm, accum_out=ssq)
                            sd = lnp.tile([128, 1], f32, tag='ln_sd')
                            nc.scalar.activation(sd, ssq, AF.Sqrt,
                                                 bias=1e-5, scale=1.0 / C)
                            rstd = lnp.tile([128, 1], f32, tag='ln_rstd')
                            nc.vector.reciprocal(rstd, sd)
                            z = lnp.tile([128, C], f32, tag=f'ln_z{comp}')
                            nc.vector.tensor_scalar(
                                z, x_ap, nm, rstd, op0=ALU.add, op1=ALU.mult)
                            for j in range(2):
                                nc.tensor.transpose(
                                    tb[:, (ci * 2 + j) * 128:(ci * 2 + j + 1) * 128],
                                    z[:, j * 128:(j + 1) * 128], ident)
                        nc.any.tensor_copy(
                            zcat[:, :, t * 128:(t + 1) * 128],
                            tb[:].rearrange('p (s q) -> p s q', s=4))

                    # ---- qkv channel-major sections (qm, qs, k)
                    sec = {}
                    for si, name in enumerate(('qm', 'qs', 'k')):
                        st = qkvp.tile([128, 4, BLK], f32, tag=f'sec_{name}')
                        sec[name] = st
                        for hp in range(4):
                            osl = si * 4 + hp
                            ps = psMM.tile([128, 512], f32, tag='mm')
                            for ks in range(4):
                                nc.tensor.matmul(
                                    ps[:],
                                    wbig_s[:, ks, osl * 128:(osl + 1) * 128],
                                    zcat[:, ks, :],
                                    start=(ks == 0), stop=(ks == 3))
                            nc.scalar.activation(
                                st[:, hp, :], ps[:], AF.Identity,
                                bias=bbig_s[:, osl:osl + 1])

                    # ---- v token-major [128 tok, 768 (h:(eineg,er,ei))]
                    vtok = []
                    for t in range(NT):
                        vt = vp.tile([128, 768], f32, tag='vtok')
                        vtok.append(vt)
                        psa = psMM.tile([128, 512], f32, tag='mm')
                        psb = psMM.tile([128, 512], f32, tag='mm')
                        for ks in range(4):
                            lhs = zcat[:, ks, t * 128:(t + 1) * 128]
                            nc.tensor.matmul(psa[:], lhs, wv_s[:, ks, 0:512],
                                             start=(ks == 0), stop=(ks == 3))
                            nc.tensor.matmul(psb[:, 0:256], lhs,
                                             wv_s[:, ks, 512:768],
                                             start=(ks == 0), stop=(ks == 3))
                        nc.vector.tensor_add(vt[:, 0:512], psa[:],
                                             bvbc_s[:, 0:512])
                        nc.vector.tensor_add(vt[:, 512:768], psb[:, 0:256],
                                             bvbc_s[:, 512:768])

                    # ---- attention per window
                    for p in range(PPB if A_LEVEL >= 2 else 0):
                        parity = p % 2
                        u2r = u2[('r', parity)]
                        u2i = u2[('i', parity)]
                        for half in range(2):
                            w = p * 2 + half
                            c0 = w * 64  # block-local token offset of window
                            # per-head full-bank psum tiles (matmul psum dst
                            # must sit at a bank start on HW); gather+bias
                            # into SBUF [64, (h q)] immediately.
                            arp = smp.tile([64, 512], f32, tag='arp')
                            aip = smp.tile([64, 512], f32, tag='aip')
                            for h in range(H):
                                hp, off = h // 2, (h % 2) * 64
                                ksl = sec['k'][off:off + 64, hp, c0:c0 + 64]
                                hs = slice(h * 64, (h + 1) * 64)
                                bank_r = psSC.tile([64, 512], f32, tag='sc',
                                                   name=f'scr{blk}_{w}_{h}')
                                nc.tensor.matmul(
                                    bank_r[:, 0:64], ksl,
                                    sec['qm'][off:off + 64, hp, c0:c0 + 64],
                                    start=True, stop=True)
                                bank_i = psSC.tile([64, 512], f32, tag='sc',
                                                   name=f'sci{blk}_{w}_{h}')
                                nc.tensor.matmul(
                                    bank_i[:, 0:64], ksl,
                                    sec['qs'][off:off + 64, hp, c0:c0 + 64],
                                    start=True, stop=True)
                                nc.vector.tensor_add(
                                    arp[:, hs], bank_r[:, 0:64], btr_s[:, hs])
                                nc.vector.tensor_add(
                                    aip[:, hs], bank_i[:, 0:64], bti_s[:, hs])
                            # phase softmax (transposed layout [k, (h q)])
                            sqr = smp1.tile([64, 512], f32, tag='sqr')
                            nc.scalar.activation(sqr, arp, AF.Square)
                            sqi = smp1.tile([64, 512], f32, tag='sqi')
                            nc.scalar.activation(sqi, aip, AF.Square)
                            m2 = smp1.tile([64, 512], f32, tag='m2')
                            nc.vector.tensor_add(m2, sqr, sqi)
                            mag = smp1.tile([64, 512], f32, tag='mag')
                            nc.scalar.activation(mag, m2, AF.Sqrt, bias=1e-18)
                            e = smp.tile([64, 512], f32, tag='e')
                            nc.scalar.activation(e, mag, AF.Exp)
                            s1 = psSM.tile([1, 512], f32, tag='sm')
                            nc.tensor.matmul(s1[:], ones64[:], e[:],
                                             start=True, stop=True)
                            s1s = smp.tile([1, 512], f32, tag='s1s')
                            nc.scalar.copy(s1s, s1[:])
                            sbc = psSM.tile([64, 512], f32, tag='sm')
                            nc.tensor.matmul(sbc[:], ones1[:], s1s[:],
                                             start=True, stop=True)
                            t2 = smp1.tile([64, 512], f32, tag='t2')
                            nc.vector.tensor_mul(t2, mag, sbc[:])
                            inv2 = smp1.tile([64, 512], f32, tag='inv2')
                            nc.vector.reciprocal(inv2, t2)
                            g2 = smp1.tile([64, 512], f32, tag='g2')
                            nc.vector.tensor_mul(g2, e, inv2)
                            # U^T writes into block-diagonal slices
                            po = half * 64
                            arp3 = arp[:].rearrange('p (h q) -> p h q', h=8)
                            aip3 = aip[:].rearrange('p (h q) -> p h q', h=8)
                            g23 = g2[:].rearrange('p (h q) -> p h q', h=8)
                            nc.vector.tensor_mul(
                                u2r[po:po + 64, :, po:po + 64], arp3, g23)
                            nc.vector.tensor_mul(
                                u2i[po:po + 64, :, po:po + 64], aip3, g23)

                        # ---- attention output, token-major [128 q, 512]
                        if A_LEVEL < 3:
                            continue
                        aos = attp.tile([128, 512], f32, tag='aos')
                        for h in range(H):
                            vs = vtok[p]
                            aoh = psSC.tile([128, 64], f32, tag='sc',
                                            name=f'ao{blk}_{p}_{h}')
                            nc.tensor.matmul(
                                aoh[:],
                                u2r[:, h, :],
                                vs[:, h * 96 + 32:h * 96 + 96],
                                start=True, stop=False)
                            nc.tensor.matmul(
                                aoh[:],
                                u2i[:, h, :],
                                vs[:, h * 96:h * 96 + 64],
                                start=False, stop=True)
                            nc.any.tensor_copy(
                                aos[:, h * 64:(h + 1) * 64], aoh[:])
                        # transpose to channel-major
                        tb2 = psMM.tile([128, 512], f32, tag='mm')
                        for j in range(4):
                            nc.tensor.transpose(
                                tb2[:, j * 128:(j + 1) * 128],
                                aos[:, j * 128:(j + 1) * 128], ident)
                        if p == 0:
                            aot = aotp.tile([128, 4, BLK], f32r, tag='aot')
                        nc.any.tensor_copy(
                            aot[:, :, p * 128:(p + 1) * 128],
                            tb2[:].rearrange('p (s q) -> p s q', s=4))

                    # ---- proj (token-major out) + bias + residual -> x2
                    for t in range(NT if A_LEVEL >= 3 else 0):
                        yps = psMM.tile([128, 512], f32, tag='mm')
                        for ks in range(4):
                            nc.tensor.matmul(
                                yps[:],
                                aot[:, ks, t * 128:(t + 1) * 128],
                                wproj_s[:, ks, :],
                                start=(ks == 0), stop=(ks == 3))
                        for ci, (comp, x2d) in enumerate(
                                (('r', x2r_d), ('i', x2i_d))):
                            tmp = attp.tile([128, C], f32, tag='x2tmp')
                            nc.vector.tensor_add(
                                tmp, yps[:, ci * 256:(ci + 1) * 256],
                                pbbc_s[:, ci * 256:(ci + 1) * 256])
                            x2sb = attp.tile([128, C], f32, tag='x2sb')
                            nc.vector.tensor_add(x2sb, tmp,
                                                 x_t[comp][:, t, :])
                            nc.sync.dma_start(
                                x2d[t0 + t * 128:t0 + (t + 1) * 128, :],
                                x2sb[:])

            # ---------------- phase B (MLP) ----------------
            if PHASE_B:
              with (
                tc.tile_pool(name='wB', bufs=1) as wB,
                tc.tile_pool(name='xp2', bufs=2) as xp2,
                tc.tile_pool(name='lnp2', bufs=3) as lnp2,
                tc.tile_pool(name='zc2', bufs=2) as zcp2,
                tc.tile_pool(name='hp', bufs=3) as hp_,
                tc.tile_pool(name='ob', bufs=4) as obp,
                tc.tile_pool(name='psF', bufs=2, space='PSUM') as psF,
                tc.tile_pool(name='psY2', bufs=4, space='PSUM') as psY2,
            ):
                wfc1_s = wB.tile([128, 4, 2048], f32r, tag='wfc1')
                nc.sync.dma_start(wfc1_s[:], wfc1_d[:].rearrange('p (k o) -> p k o', k=4).bitcast(f32r))
                b1_s = wB.tile([128, 16], f32, tag='b1')
                nc.sync.dma_start(b1_s[:], b1_d[:])
                b1s_s = wB.tile([128, 16], f32, tag='b1s')
                nc.sync.dma_start(b1s_s[:], b1s_d[:])
                wfc2_s = wB.tile([128, 16, 512], f32r, tag='wfc2')
                nc.sync.dma_start(wfc2_s[:], wfc2_d[:].rearrange('p (k o) -> p k o', k=16).bitcast(f32r))
                b2bc_s = wB.tile([128, 512], f32, tag='b2bc')
                nc.sync.dma_start(b2bc_s[:], b2bc_d[:])

                for blk in range(NBLK):
                    t0 = blk * BLK
                    x2_t = {}
                    for comp, x2d in (('r', x2r_d), ('i', x2i_d)):
                        xt = xp2.tile([128, NT, C], f32, tag=f'x2{comp}')
                        nc.sync.dma_start(
                            xt[:], x2d[t0:t0 + BLK, :].rearrange(
                                '(t p) c -> p t c', p=128))
                        x2_t[comp] = xt
                    z2cat = zcp2.tile([128, 4, BLK], f32r, tag='z2cat')
                    for t in range(NT):
                        tb = psF.tile([128, 512], f32, tag='f')
                        for ci, comp in enumerate(('r', 'i')):
                            x_ap = x2_t[comp][:, t, :]
                            ssum = lnp2.tile([128, 1], f32, tag='ln_s')
                            nc.vector.reduce_sum(out=ssum, in_=x_ap, axis=AX.X)
                            nm = lnp2.tile([128, 1], f32, tag='ln_nm')
                            nc.vector.tensor_scalar_mul(nm, ssum, -1.0 / C)
                            sq = lnp2.tile([128, C], f32, tag='ln_sq')
                            ssq = lnp2.tile([128, 1], f32, tag='ln_ssq')
                            nc.scalar.activation(sq, x_ap, AF.Square,
                                                 bias=nm, accum_out=ssq)
                            sd = lnp2.tile([128, 1], f32, tag='ln_sd')
                            nc.scalar.activation(sd, ssq, AF.Sqrt,
                                                 bias=1e-5, scale=1.0 / C)
                            rstd = lnp2.tile([128, 1], f32, tag='ln_rstd')
                            nc.vector.reciprocal(rstd, sd)
                            z = lnp2.tile([128, C], f32, tag=f'ln_z{comp}')
                            nc.vector.tensor_scalar(
                                z, x_ap, nm, rstd, op0=ALU.add, op1=ALU.mult)
                            for j in range(2):
                                nc.tensor.transpose(
                                    tb[:, (ci * 2 + j) * 128:(ci * 2 + j + 1) * 128],
                                    z[:, j * 128:(j + 1) * 128], ident)
                        nc.any.tensor_copy(
                            z2cat[:, :, t * 128:(t + 1) * 128],
                            tb[:].rearrange('p (s q) -> p s q', s=4))

                    # fc1 (ch-major, Lrelu evac) interleaved with fc2 accum
                    y2ps = [psY2.tile([128, 512], f32, tag='y2',
                                      name=f'y2ps_{blk}_{t}')
                            for t in range(NT)]
                    for j in range(16):
                        fps = psF.tile([128, 512], f32, tag='f')
                        for ks in range(4):
                            nc.tensor.matmul(
                                fps[:],
                                wfc1_s[:, ks, j * 128:(j + 1) * 128],
                                z2cat[:, ks, :],
                                start=(ks == 0), stop=(ks == 3))
                        slope = slope_r if j < 8 else slope_i
                        a_t = hp_.tile([128, 512], f32, tag='pre')
                        nc.scalar.activation(a_t[:], fps[:], AF.Identity,
                                             bias=b1_s[:, j:j + 1])
                        r_t = hp_.tile([128, 512], f32, tag='rel')
                        nc.scalar.activation(r_t[:], fps[:], AF.Relu,
                                             bias=b1s_s[:, j:j + 1],
                                             scale=1.0 - slope)
                        hj = hp_.tile([128, 512], f32r, tag='hj')
                        nc.vector.scalar_tensor_tensor(
                            hj[:], a_t[:], slope, r_t[:],
                            op0=ALU.mult, op1=ALU.add)
                        for t in range(NT):
                            nc.tensor.matmul(
                                y2ps[t][:],
                                hj[:, t * 128:(t + 1) * 128],
                                wfc2_s[:, j, :],
                                start=(j == 0), stop=(j == 15))
                    for t in range(NT):
                        for ci, (comp, od) in enumerate(
                                (('r', outr_d), ('i', outi_d))):
                            tmp = obp.tile([128, C], f32, tag='otmp')
                            nc.vector.tensor_add(
                                tmp, y2ps[t][:, ci * 256:(ci + 1) * 256],
                                b2bc_s[:, ci * 256:(ci + 1) * 256])
                            osb = obp.tile([128, C], f32, tag='osb')
                            nc.vector.tensor_add(osb, tmp, x2_t[comp][:, t, :])
                            nc.sync.dma_start(
                                od[t0 + t * 128:t0 + (t + 1) * 128, :], osb[:])

    nc.compile()
    return nc


_CACHE = {}


def _get_program(slope_r, slope_i):
    key = (slope_r, slope_i)
    if key not in _CACHE:
        _CACHE[key] = build_program(slope_r, slope_i)
    return _CACHE[key]


def _run(inputs, trace=False):
    from concourse.bass_utils import run_bass_kernel_spmd

    pk = build_packs(inputs)
    nc = _get_program(pk['slope_r'], pk['slope_i'])
    xr = np.asarray(inputs['x_real'], np.float32)
    xi = np.asarray(inputs['x_imag'], np.float32)
    shared = {k: pk[k] for k in ('wbig', 'bbig', 'wv', 'bvbc', 'btr', 'bti',
                                 'wproj', 'pbbc', 'wfc1', 'b1', 'b1s',
                                 'wfc2', 'b2bc')}
    in_maps = []
    for b in range(B):
        m = dict(shared)
        m['xr'] = np.ascontiguousarray(xr[b])
        m['xi'] = np.ascontiguousarray(xi[b])
        in_maps.append(m)
    try:
        res = run_bass_kernel_spmd(nc, in_maps, core_ids=list(range(B)),
                                   trace=trace)
    except ModuleNotFoundError:
        # NTFF profiling hook unavailable in this container; run untraced.
        res = run_bass_kernel_spmd(nc, in_maps, core_ids=list(range(B)),
                                   trace=False)
    out = np.empty((2, B, N, C), np.float32)
    for b in range(B):
        out[0, b] = res.results[b]['outr']
        out[1, b] = res.results[b]['outi']
    return out, res.exec_time_ns


def kernel(**inputs):
    out, _ = _run(inputs)
    return out
